# revision 59
# baseline (speedup 1.0000x reference)
"""AttentativeResidual Trainium2 kernel.

out[b,t,n,:] = x[b,t,n,:] + softmax_m(x[b,t,n,:] @ Wq @ Wk^T @ rs[b]^T) @ (rs[b] @ Wv)

Shapes: x [4,8,2048,128], residual_source [4,2048,128], W* [128,128], fp32.

Sharding: data-parallel over (b,t): core i handles b = i//2, t in
[(i%2)*4, (i%2)*4+4). Each core sees one batch b.

Host-side prep (all tiny or memory-layout-only):
  GT = (Wq @ Wk^T) @ rs[b]^T  [c,m] fp16  -- folds both projections, so
       on-device logits^T[m,n] = GT_m^T @ xT need a single matmul chain.
  V  = rs[b] @ Wv  [m,c] bf16 -- the AV moving operand; device appends a
       ones column (softmax denominator).
  xT16 = x[b,t]^T  [c,n] fp16 -- pre-transposed on host, so the PE does
       ZERO transposes in steady state (the old kernel spent ~8k PE
       cycles/core transposing x on-device).
  x fp32 stays [n,c] for the residual-add epilogue only.

Device algorithm per core, per (t, quarter qq), m-slot 0..15:
  one 512-col fp16 aff matmul aff^T[m-part, n] = GT_m^T @ xT into a
  4-deep rotation of 1-bank psum tiles; exp -> bf16 ea tile, split
  ACT 9 / DVE 7 (GPSIMD cannot touch PSUM). AV is software-pipelined
  with a one-quarter skew so PE never idles: slots 10-15 start THIS
  quarter's chunks 0,1 (pieces 0-2, psC) as their eas land; slots 0-9
  of the NEXT quarter close them (piece 3 + epilogue) and run chunks
  2,3 (psB). Per r-chunk: av[r,129] = sum_m ea[m][:,r]^T @ v_aug[m]
  (bf16, psum-accumulated); out[r,:] = av[:,:128] * (1/av[:,128]) +
  x[r,:] in ONE fused DVE pass (AFFINE_THEN_ADD). The final quarter
  accumulates all four of its own chunks inline (psC + psB freed by
  slot 9) so the drain tail is just closing accumulations, epilogues,
  and two small output DMAs.

Startup: all setup DMAs ride the SP HWDGE queue in first-use order
(descriptor generation and the transfers each serialize on single
global devices, so chunk granularity/order is tuned so every consumer
finds its data just in time). The ACT queue stays clear so
LoadActFuncSet + exps start immediately; DVE queue stays clear for
exps; Pool only does the ones-column memset.

exp is computed without max-subtraction: logits ~ N(0, 128), |l| < ~75
with overwhelming probability, exp fits fp32/bf16 range. ea/v in bf16
because unnormalized exp(l) overflows fp16.

Fast exp (DVE tiles): exp(l) ~= bitcast_bf16(int16(l * 128*log2e +
127*128 + c)) via a single dual-op tensor_scalar whose fp32->int16
output conversion truncates (the +0.5 is folded into c). The int16 bit
pattern IS the bf16 exponent+mantissa of 2^(l*log2e) with the mantissa
linearly interpolated (~+-4% sawtooth). Softmax weights are ratios, so
the common-mode part cancels; end-to-end rel err ~1e-2 vs the 2e-2 gate.

PE work per (t,qq): 16x512 aff (8192cyc) + 16x4x129 AV (8256cyc) =
~6.86us at 2.4GHz = the per-quarter window; ACT 9 exps ~5.5us; DVE
7 exps + epilogues ~5.7us. PE ~91% busy over the 120.1us total.
"""
import numpy as np

import concourse.bacc as bacc
import concourse.tile as tile
import concourse.mybir as mybir
from concourse.dve_ops import AFFINE_THEN_ADD

F32 = mybir.dt.float32
F16 = mybir.dt.float16
I16 = mybir.dt.int16
BF16 = mybir.dt.bfloat16
EXP = mybir.ActivationFunctionType.Exp
MULT = mybir.AluOpType.mult
ADD = mybir.AluOpType.add

B, T, N, C = 4, 8, 2048, 128
NCORES = 8
TPC = (B * T) // NCORES          # (b,t) pairs per core = 4
NT = N // 128                    # 16 key tiles

# Schraudolph fast-exp constants (bf16 bit pattern via int16 convert).
EXP_A = float(128.0 * np.log2(np.e))
EXP_B = float(127.0 * 128.0 + 0.5 - 6.0)
# m-slot -> exp engine: 'A' = ACT true exp, 'D' = DVE fast-exp.
# 9 ACT / 7 DVE balances ACT (612ns/exp) against DVE (658ns/exp +
# epilogue) under the ~6.9us/quarter PE window.
EXP_ENG = ['A'] * NT
for _i in (1, 3, 5, 7, 9, 11, 13):
    EXP_ENG[_i] = 'D'


def _body(ctx, tc, xs, xt16, gt, vb, hdr, out):
    nc = tc.nc
    const = ctx.enter_context(tc.tile_pool(name="const", bufs=1))
    xpool = ctx.enter_context(tc.tile_pool(name="xp", bufs=2))
    xtp = ctx.enter_context(tc.tile_pool(name="xtp", bufs=2))
    eap = ctx.enter_context(tc.tile_pool(name="eap", bufs=36))
    outp = ctx.enter_context(tc.tile_pool(name="outp", bufs=3))
    recp = ctx.enter_context(tc.tile_pool(name="recp", bufs=8))
    # psA: 4 x 1-bank aff tiles -- the 4-deep rotation halves the
    # aff->exp->aff WAR chain latency vs 2 x [128,1024].
    psA = ctx.enter_context(tc.tile_pool(name="psA", bufs=4, space="PSUM"))
    psB = ctx.enter_context(tc.tile_pool(name="psB", bufs=2, space="PSUM"))
    psC = ctx.enter_context(tc.tile_pool(name="psC", bufs=2, space="PSUM"))

    xr = xs[:, :, :].rearrange("t (i p) c -> t p i c", p=128)
    outr = out[:, :, :].rearrange("t (i p) c -> t p i c", p=128)

    # All setup DMAs ride the SP (sync) HWDGE queue in first-use order.
    # Descriptor generation (625ns each) and the transfers serialize on
    # single global devices, so chunk sizes/order are tuned so each
    # consumer finds its data just in time: gt slots 0-1 + xT(0) quarter
    # 0 gate the first affs (~3.3us); v slots 0-11 land before quarter
    # 0's own-AV slots (~7us); x fp32 slots 0-3 before the first
    # epilogues (~9.4us). The ACT queue stays clear so LoadActFuncSet +
    # exps start immediately; the DVE queue stays clear for exps; Pool
    # (no HWDGE -- engine-executed DMAs are slow) only does the
    # ones-column memset.
    gt_sb = const.tile([128, NT, 128], F16, tag="gt")
    v_aug = const.tile([128, NT, 129], BF16, tag="vaug")
    gtr = gt[:, :].rearrange("c (i m) -> c i m", m=128)
    vbr = vb[:, :].rearrange("(i p) c -> p i c", p=128)
    nc.gpsimd.memset(v_aug[:, :, 128:129], 1.0)

    xts = {}
    xss = {}
    xts[0] = xtp.tile([128, N], F16, tag="xt", name="xt_sb")
    xss[0] = xpool.tile([128, NT, 128], F32, tag="x", name="x_sb")
    # hdr packs gt slots 0-1 + xT(0) quarter-0 cols in ONE DMA: a single
    # 625ns descriptor generation gates the first aff, and every later
    # setup DMA shifts one generation earlier on the serialized HWDGE.
    hdr_sb = const.tile([128, 768], F16, tag="hdr")
    nc.sync.dma_start(out=hdr_sb, in_=hdr[:, :])
    nc.sync.dma_start(out=gt_sb[:, 2:16, :], in_=gtr[:, 2:16, :])
    nc.sync.dma_start(out=v_aug[:, 0:12, 0:128], in_=vbr[:, 0:12, :])
    nc.sync.dma_start(out=xss[0][:, 0:4, :], in_=xr[0][:, 0:4, :])
    nc.sync.dma_start(out=xts[0][:, 512:1024], in_=xt16[0][:, 512:1024])
    nc.sync.dma_start(out=v_aug[:, 12:16, 0:128], in_=vbr[:, 12:16, :])
    nc.sync.dma_start(out=xts[0][:, 1024:2048], in_=xt16[0][:, 1024:2048])
    nc.sync.dma_start(out=xss[0][:, 4:16, :], in_=xr[0][:, 4:16, :])

    def issue_xt_dma(t, split=1):
        # fp16 pre-transposed x (host prep) feeds the aff matmuls
        # directly -- per-partition-contiguous 4KB lines.
        xts[t] = xtp.tile([128, N], F16, tag="xt", name="xt_sb")
        step = N // split
        for j in range(split):
            nc.sync.dma_start(out=xts[t][:, step * j:step * (j + 1)],
                              in_=xt16[t][:, step * j:step * (j + 1)])

    def issue_x_dma(t):
        xss[t] = xpool.tile([128, NT, 128], F32, tag="x", name="x_sb")
        nc.sync.dma_start(out=xss[t], in_=xr[t])


    # One AV r-chunk of a previous QUARTER: 16 accumulating bf16 matmuls
    # split into four 4-matmul pieces (one per m-slot) so PE work stays
    # evenly paced, then out = av * (1/denominator) + x in one fused DVE
    # pass.
    av_live = {}

    def emit_av_piece(ph, k, piece):
        t_, qq_, eas, x_sb_, out_sb = ph
        if piece == 0:
            av_live[k] = psB.tile([128, 129], F32, tag="av", name="av")
        av = av_live[k]
        for m in range(4 * piece, 4 * piece + 4):
            nc.tensor.matmul(av, eas[m][:, 128 * k:128 * (k + 1)],
                             v_aug[:, m, :],
                             start=(m == 0), stop=(m == NT - 1))
        if piece == 3:
            emit_epilogue(ph, k, av)

    def emit_epilogue(ph, k, av):
        t_, qq_, eas, x_sb_, out_sb = ph
        rec = recp.tile([128, 1], F32, tag="rec")
        nc.vector.reciprocal(out=rec, in_=av[:, 128:129])
        nc.vector._custom_dve(AFFINE_THEN_ADD, out=out_sb[:, k, :],
                              in0=av[:, 0:128],
                              in1=x_sb_[:, 4 * qq_ + k, :],
                              s0=rec, s1=0.0)
        if k == 3:
            nc.sync.dma_start(out=outr[t_][:, 4 * qq_:4 * qq_ + 4, :],
                              in_=out_sb)

    def emit_exp(eng, ea_half, ap):
        if eng == 'A':
            nc.scalar.activation(out=ea_half, in_=ap, func=EXP)
        else:
            nc.vector.tensor_scalar(out=ea_half.bitcast(I16), in0=ap,
                                    scalar1=EXP_A, scalar2=EXP_B,
                                    op0=MULT, op1=ADD)

    prev = None
    for t in range(TPC):
        if t + 1 < TPC:
            issue_xt_dma(t + 1)
            issue_x_dma(t + 1)
        for qq in range(4):
            eas_h = []
            own_avs = {}
            cur = None
            last_q = (t == TPC - 1 and qq == 3)
            for m in range(NT):
                ea = eap.tile([128, 512], BF16, tag="ea")
                ap = psA.tile([128, 512], F32, tag="aff", name="ap")
                gsrc = (hdr_sb[:, 128 * m:128 * (m + 1)] if m < 2
                        else gt_sb[:, m, :])
                msrc = (hdr_sb[:, 256:768] if (t == 0 and qq == 0)
                        else xts[t][:, 512 * qq:512 * (qq + 1)])
                nc.tensor.matmul(ap, gsrc, msrc, start=True, stop=True)
                # Quarter 0 runs 8 exps on DVE (no epilogue load yet):
                # its ACT chain gates the q0->q1 psA rotation.
                eng = EXP_ENG[m]
                if prev is None and m in (12, 13):
                    eng = 'D'
                emit_exp(eng, ea, ap)
                eas_h.append(ea)
                # Previous quarter's 10 leftover AV pieces in slots 0-9:
                # chunk0/1 piece3 (closing the psC accumulations started
                # there), then chunks 2,3 in psB.
                if prev is not None and m < 10:
                    t_, qq_, eas, x_sb_, out_sb, pavs = prev
                    if m < 2:
                        av = pavs[m]
                        for j in range(12, 16):
                            nc.tensor.matmul(
                                av, eas[j][:, 128 * m:128 * (m + 1)],
                                v_aug[:, j, :], start=False,
                                stop=(j == NT - 1))
                        emit_epilogue(prev[:5], m, av)
                    else:
                        emit_av_piece(prev[:5], 2 + (m - 2) // 4,
                                      (m - 2) % 4)
                # Own chunks 0,1 pieces 0-2 in slots 10-15 (psC): fills
                # PE while this quarter's exps drain; the next quarter
                # closes them with piece 3 and the epilogue.
                if m >= 10:
                    s = m - 10
                    k, p = s % 2, s // 2
                    if p == 0:
                        own_avs[k] = psC.tile([128, 129], F32,
                                              tag="misc", name="avo")
                    av = own_avs[k]
                    for j in range(4 * p, 4 * p + 4):
                        nc.tensor.matmul(
                            av, eas_h[j][:, 128 * k:128 * (k + 1)],
                            v_aug[:, j, :], start=(j == 0), stop=False)
                if last_q and m >= 10:
                    # Final quarter: chunks 2,3 also accumulate inline
                    # in psB (freed by slot 9) so the post-loop drain is
                    # just the closing accumulations and epilogues.
                    for k in range(2, 4):
                        if m == 10:
                            if k == 2:
                                tail_avs = {}
                            tail_avs[k] = psB.tile([128, 129], F32,
                                                   tag="av", name="avt")
                        for j in (2 * (m - 10), 2 * (m - 10) + 1):
                            nc.tensor.matmul(
                                tail_avs[k],
                                eas_h[j][:, 128 * k:128 * (k + 1)],
                                v_aug[:, j, :], start=(j == 0),
                                stop=False)
            out_sb = outp.tile([128, 4, 128], F32, tag="o")
            prev = (t, qq, eas_h, xss[t], out_sb, own_avs)
    # Drain tail: chunks 0,1 close first (one accumulation each) so
    # their epilogues + 0:2 DMA machinery overlap chunks 2,3's closing
    # matmuls; only chunk 3's single epilogue + a small 2:4 DMA sit on
    # the kernel-ending chain.
    t_, qq_, eas, x_sb_, out_sb, pavs = prev
    for k in range(4):
        # Close every chunk with its eas[12..15] accumulations (chunks
        # 0,1 live in psC with pieces 0-2; chunks 2,3 in psB with pairs
        # 0-11), epilogue, and DMA out in two halves so only chunk 3's
        # chain ends the kernel.
        av = pavs[k] if k < 2 else tail_avs[k]
        for j in range(12, 16):
            nc.tensor.matmul(av, eas[j][:, 128 * k:128 * (k + 1)],
                             v_aug[:, j, :], start=False,
                             stop=(j == NT - 1))
        rec = recp.tile([128, 1], F32, tag="rec")
        nc.vector.reciprocal(out=rec, in_=av[:, 128:129])
        nc.vector._custom_dve(AFFINE_THEN_ADD, out=out_sb[:, k, :],
                              in0=av[:, 0:128],
                              in1=x_sb_[:, 4 * qq_ + k, :],
                              s0=rec, s1=0.0)
        if k == 1:
            nc.sync.dma_start(out=outr[t_][:, 4 * qq_:4 * qq_ + 2, :],
                              in_=out_sb[:, 0:2, :])
    nc.sync.dma_start(out=outr[t_][:, 4 * qq_ + 2:4 * qq_ + 4, :],
                      in_=out_sb[:, 2:4, :])


def _run_on_cores(nc, in_maps):
    """Run the bass module on len(in_maps) NeuronCores as independent
    single-device programs dispatched concurrently.

    run_bass_kernel_spmd's multi-core path lowers to one shard_map program
    spanning 8 devices, which deadlocks through the axon PJRT tunnel in this
    environment. Independent per-device jits of the same bass_exec body work
    (and still run concurrently on all 8 cores), so we dispatch those.
    """
    import jax
    from concourse import bass2jax

    bass2jax.install_neuronx_cc_hook()
    devices = jax.devices()[:len(in_maps)]
    assert len(devices) == len(in_maps)

    partition_name = (nc.partition_id_tensor.name
                      if nc.partition_id_tensor else None)
    dbg_name = nc.dbg_addr.name if nc.dbg_addr is not None else None
    in_names, out_names, out_avals, zero_outs = [], [], [], []
    for alloc in nc.m.functions[0].allocations:
        if not isinstance(alloc, mybir.MemoryLocationSet):
            continue
        name = alloc.memorylocations[0].name
        if alloc.kind == "ExternalInput":
            if name != partition_name:
                in_names.append(name)
        elif alloc.kind == "ExternalOutput":
            shape = tuple(alloc.tensor_shape)
            dtype = mybir.dt.np(alloc.dtype)
            out_names.append(name)
            out_avals.append(jax.core.ShapedArray(shape, dtype))
            zero_outs.append(np.zeros(shape, dtype))

    n_params = len(in_names)
    in_names_all = tuple(in_names + out_names + (
        [partition_name] if partition_name else []))
    donate = tuple(range(n_params, n_params + len(out_names)))

    def _bass_body(*args):
        operands = list(args)
        if partition_name is not None:
            operands.append(bass2jax.partition_id_tensor())
        outs = bass2jax._bass_exec_p.bind(
            *operands,
            out_avals=tuple(out_avals),
            in_names=in_names_all,
            out_names=tuple(out_names),
            lowering_input_output_aliases=(),
            sim_require_finite=True,
            sim_require_nnan=True,
            nc=nc,
        )
        return tuple(outs)

    jf = jax.jit(_bass_body, donate_argnums=donate, keep_unused=True)
    futs = []
    for c, im in enumerate(in_maps):
        im = dict(im)
        if dbg_name is not None:
            im[dbg_name] = np.zeros((1, 2), np.uint32)
        args = [jax.device_put(np.asarray(im[n]), devices[c])
                for n in in_names]
        args += [jax.device_put(z, devices[c]) for z in zero_outs]
        futs.append(jf(*args))
    return [{n: np.asarray(outs[i]) for i, n in enumerate(out_names)}
            for outs in futs]


_NC_CACHE = None


def _get_nc():
    global _NC_CACHE
    if _NC_CACHE is None:
        nc = bacc.Bacc("TRN2", target_bir_lowering=False)
        xs = nc.dram_tensor("xs", [TPC, N, C], F32, kind="ExternalInput")
        xt16 = nc.dram_tensor("xt16", [TPC, C, N], F16, kind="ExternalInput")
        gt = nc.dram_tensor("gt", [C, N], F16, kind="ExternalInput")
        vb = nc.dram_tensor("vb", [N, C], BF16, kind="ExternalInput")
        hdr = nc.dram_tensor("hdr", [C, 768], F16, kind="ExternalInput")
        out = nc.dram_tensor("out", [TPC, N, C], F32, kind="ExternalOutput")
        from contextlib import ExitStack
        with tile.TileContext(nc) as tc, ExitStack() as ctx:
            _body(ctx, tc, xs, xt16, gt, vb, hdr, out)
        nc.finalize()
        _NC_CACHE = nc
    return _NC_CACHE


def kernel(x, residual_source, Wq, Wk, Wv):
    import ml_dtypes
    x = np.asarray(x, dtype=np.float32)
    residual_source = np.asarray(residual_source, dtype=np.float32)
    Wq = np.asarray(Wq, dtype=np.float32)
    Wk = np.asarray(Wk, dtype=np.float32)
    Wv = np.asarray(Wv, dtype=np.float32)

    a = Wq @ Wk.T                       # logits = x @ A @ rs^T
    nc = _get_nc()

    in_maps = []
    for core in range(NCORES):
        b, toff = core // 2, (core % 2) * TPC
        rs = residual_source[b]
        xc = x[b, toff:toff + TPC]                       # [TPC, N, C]
        in_maps.append({
            "xs": np.ascontiguousarray(xc),
            "xt16": np.ascontiguousarray(
                xc.transpose(0, 2, 1).astype(np.float16)),
            "gt": np.ascontiguousarray((a @ rs.T).astype(np.float16)),
            "vb": np.ascontiguousarray(
                (rs @ Wv).astype(ml_dtypes.bfloat16)),
            "hdr": np.ascontiguousarray(np.concatenate(
                [(a @ rs.T)[:, 0:256], xc[0].T[:, 0:512]],
                axis=1).astype(np.float16)),
        })
    results = _run_on_cores(nc, in_maps)

    out = np.empty((B, T, N, C), np.float32)
    for core in range(NCORES):
        b, toff = core // 2, (core % 2) * TPC
        out[b, toff:toff + TPC] = results[core]["out"]
    return out


if __name__ == "__main__":
    rng = np.random.default_rng(0)
    x = rng.standard_normal((B, T, N, C)).astype(np.float32)
    rs = rng.standard_normal((B, N, C)).astype(np.float32)
    s = 1.0 / np.sqrt(C)
    Wq = (rng.standard_normal((C, C)) * s).astype(np.float32)
    Wk = (rng.standard_normal((C, C)) * s).astype(np.float32)
    Wv = (rng.standard_normal((C, C)) * s).astype(np.float32)
    y = kernel(x, rs, Wq, Wk, Wv)
    print("out", y.shape, y.dtype)
